# revision 19
# baseline (speedup 1.0000x reference)
"""Local causal (sliding-window) attention block on 8 TRN2 NeuronCores.

Reference computation (per batch b):
    h = LayerNorm(x) * gamma + beta
    Q = h@Wq, K = h@Wk, V = h@Wv          (heads: 16 x 64)
    S = QK^T/sqrt(dk) masked to causal band of width 256
    out = x + softmax(S)@V @ Wo + bo

Sharding: 8 cores = 2 batches x 4 head-groups (4 heads each).
Each core computes LN(x_b), its head-group's Q/K/V, banded attention,
and a partial out-projection  attn_g @ Wo[g]  (token-major, [T, D]).
Host reduces: out[b] = x[b] + sum_g partial[b,g] + bo.

Pipelined single-pass design:
  - all matmul operands bf16 (1 cyc/row on PE, FWL weight loads)
  - h^T and P^T via XBAR DMA-transpose (no PE transposes, no PSUM
    evacuation copies for transposes)
  - band mask applied multiplicatively after exp on GpSimd via
    scalar_tensor_tensor with fused denominator accumulation
  - per-supertile (512 tokens) software pipeline: LN -> QKV -> banded
    attention (lag-3 between softmax and AV) -> out-projection, so the
    PE never idles long enough to lose the HAM clock-gate warmup.
"""

import os

import numpy as np

import concourse.bass as bass
import concourse.tile as tile
from concourse import bacc, mybir
from concourse.bass_utils import run_bass_kernel_spmd

F32 = mybir.dt.float32
BF16 = mybir.dt.bfloat16

T = 2048          # tokens per batch
D = 1024          # model dim
HG = 4            # heads per core
DK = 64           # head dim
DG = HG * DK      # head-group feature width (256)
WIN = 256         # attention window
P = 128           # partitions
NT = T // P       # 16 token tiles
NST = 4           # supertiles (512 tokens each)
KC = D // P       # 8 feature chunks
LN_EPS = 1e-5
AVLAG = 6         # softmax->AV pipeline lag in query blocks

# filled by test.py via run(trace=True)
LAST_PROFILE = {}


def _body(tc):
    nc = tc.nc

    x = nc.dram_tensor("x", [T, D], BF16, kind="ExternalInput").ap()
    wq = nc.dram_tensor("wq", [D, DG], BF16, kind="ExternalInput").ap()
    wk = nc.dram_tensor("wk", [D, DG], BF16, kind="ExternalInput").ap()
    wv = nc.dram_tensor("wv", [D, DG], BF16, kind="ExternalInput").ap()
    wo = nc.dram_tensor("wo", [DG, D], BF16, kind="ExternalInput").ap()
    bq = nc.dram_tensor("bq", [P, DG // P], F32, kind="ExternalInput").ap()
    bk = nc.dram_tensor("bk", [P, DG // P], F32, kind="ExternalInput").ap()
    bv = nc.dram_tensor("bv", [P, DG], F32, kind="ExternalInput").ap()
    m01 = nc.dram_tensor("m01", [P, 3 * 3 * P], BF16, kind="ExternalInput").ap()
    partial = nc.dram_tensor("partial", [T, D], BF16, kind="ExternalOutput").ap()

    with (
        tc.tile_pool(name="consts", bufs=1) as consts,
        tc.tile_pool(name="big", bufs=1) as big,
    ):
        # ---- resident SBUF tensors ----
        wq_sb = consts.tile([P, KC, DG], BF16, tag="wq")
        wk_sb = consts.tile([P, KC, DG], BF16, tag="wk")
        wv_sb = consts.tile([P, KC, DG], BF16, tag="wv")
        wo_sb = consts.tile([P, DG // P, D], BF16, tag="wo")
        bq_sb = consts.tile([P, DG // P], F32, tag="bq")
        bk_sb = consts.tile([P, DG // P], F32, tag="bk")
        bv_sb = consts.tile([P, DG], F32, tag="bv")
        m01_sb = consts.tile([P, 3, 3 * P], BF16, tag="m01")
        eps_sb = consts.tile([P, 1], F32, tag="eps")

        nc.sync.dma_start(out=wv_sb, in_=wv.rearrange("(c p) n -> p c n", p=P))
        nc.sync.dma_start(out=wq_sb, in_=wq.rearrange("(c p) n -> p c n", p=P))
        nc.sync.dma_start(out=wk_sb, in_=wk.rearrange("(c p) n -> p c n", p=P))
        nc.scalar.dma_start(out=wo_sb, in_=wo.rearrange("(c p) n -> p c n", p=P))
        nc.sync.dma_start(out=bq_sb, in_=bq)
        nc.sync.dma_start(out=bk_sb, in_=bk)
        nc.sync.dma_start(out=bv_sb, in_=bv)
        nc.sync.dma_start(out=m01_sb, in_=m01.rearrange("p (c n) -> p c n", c=3))
        nc.vector.memset(eps_sb, LN_EPS)

        # feature-major Q^T/K^T (scaled; bias folded), token-major V,
        # all bf16, resident for the whole batch
        qt_sb = big.tile([P, DG // P, T], BF16, tag="qt")
        kt_sb = big.tile([P, DG // P, T], BF16, tag="kt")
        v_sb = big.tile([P, NT, DG], BF16, tag="v")

        with (
            tc.tile_pool(name="xp", bufs=2) as xp,
            tc.tile_pool(name="hnp", bufs=4) as hnp,
            tc.tile_pool(name="lnst", bufs=4) as lnst,
            tc.tile_pool(name="rsp", bufs=8) as rsp,
            tc.tile_pool(name="htp", bufs=2) as htp,
            tc.tile_pool(name="otp", bufs=2) as otp,
            tc.tile_pool(name="obp", bufs=2) as obp,
            tc.tile_pool(name="pbp", bufs=3) as pbp,
            tc.tile_pool(name="ptsp", bufs=8) as ptsp,
            tc.tile_pool(name="denp", bufs=4) as denp,
            tc.tile_pool(name="s2p", bufs=3, space="PSUM") as s2p,
            tc.tile_pool(name="mmp", bufs=2, space="PSUM") as mmp,
        ):
            ht = {}      # st -> ht tile [P, 4, KC, P]
            s2t = {}     # (qb, pair) -> psum tile
            pbt = {}     # qb -> [P, 4*3*P] bf16
            ptst = {}    # qb -> [P, 12, P] bf16
            dent = {}    # qb -> ([P,4] den, [P,4] recip)
            avt = {}     # qb -> [P, 2, P] psum
            ott = {}     # st -> [P, 2, 512] bf16

            x4t = {}

            def emit_ln_x(st):
                """Allocate ht + load the supertile's x in one DMA."""
                ht[st] = htp.tile([P, 4, KC, P], BF16, tag="ht", name="ht")
                x4 = xp.tile([P, 4, D], BF16, tag="x4", name="x4")
                x4t[st] = x4
                nc.scalar.dma_start(
                    out=x4, in_=x.rearrange("(s t p) d -> p (s t) d", p=P, t=4)[
                        :, st * 4:(st + 1) * 4, :])

            def emit_ln_tile(st, t):
                """LN stats + apply + h^T transpose for ONE token tile, so
                LN work spreads across query blocks and never bunches up
                ahead of the softmax exps in the ACT queue."""
                if True:
                    tb = st * 4 + t
                    xt = x4t[st][:, t, :]

                    stats = lnst.tile([P, 2, 6], F32, tag="stats")
                    xg = xt.rearrange("p (g d) -> p g d", g=2)
                    nc.vector.bn_stats(out=stats[:, 0, :], in_=xg[:, 0, :])
                    nc.vector.bn_stats(out=stats[:, 1, :], in_=xg[:, 1, :])
                    mv = lnst.tile([P, 2], F32, tag="mv")
                    nc.vector.bn_aggr(out=mv, in_=stats)

                    rstd = rsp.tile([P, 1], F32, tag="rstd", name="rstd")
                    nc.scalar.activation(
                        out=rstd, in_=mv[:, 1:2],
                        func=mybir.ActivationFunctionType.Sqrt,
                        bias=eps_sb, scale=1.0,
                    )
                    nc.vector.reciprocal(out=rstd, in_=rstd)
                    nmr = rsp.tile([P, 1], F32, tag="nmr", name="nmr")
                    nc.vector.scalar_tensor_tensor(
                        out=nmr, in0=mv[:, 0:1], scalar=-1.0, in1=rstd,
                        op0=mybir.AluOpType.mult, op1=mybir.AluOpType.mult)

                    hn = hnp.tile([P, D], BF16, tag="hn")
                    nc.scalar.activation(
                        out=hn, in_=xt,
                        func=mybir.ActivationFunctionType.Identity,
                        bias=nmr, scale=rstd,
                    )
                    nc.sync.dma_start_transpose(out=ht[st][:, t, :, :], in_=hn)

            def emit_qkv(st):
                """Q/K (feature-major) and V (token-major) projections for
                supertile st from ht[st]."""
                h = ht[st]
                tsl = slice(st * 512, (st + 1) * 512)
                for t in range(4):
                    tb = st * 4 + t
                    ps = mmp.tile([P, 512], F32, tag="mm", name="mm")
                    for kc in range(KC):
                        nc.tensor.matmul(
                            ps[:, 0:DG],
                            h[:, t, kc, :],
                            wv_sb[:, kc, :],
                            start=(kc == 0), stop=(kc == KC - 1),
                        )
                    nc.vector.tensor_add(v_sb[:, tb, :], ps[:, 0:DG], bv_sb)
                for w_sb, dst, b_sb in ((wq_sb, qt_sb, bq_sb),
                                        (wk_sb, kt_sb, bk_sb)):
                    for oc in range(DG // P):
                        ps = mmp.tile([P, 512], F32, tag="mm", name="mm")
                        for kc in range(KC):
                            nc.tensor.matmul(
                                ps,
                                w_sb[:, kc, oc * P:(oc + 1) * P],
                                h[:, :, kc, :],
                                start=(kc == 0), stop=(kc == KC - 1),
                            )
                        nc.vector.tensor_scalar_add(
                            dst[:, oc, tsl], ps, b_sb[:, oc:oc + 1])

            def emit_qk_softmax(qb):
                """Banded QK^T, exp, multiplicative band mask + denominator
                (GpSimd), normalization, and DMA-transpose of P."""
                njb = min(3, qb + 1)
                jw = njb * P
                j0 = (qb - njb + 1) * P
                cls = min(qb, 2)
                qsl = slice(qb * P, (qb + 1) * P)
                # 12 P blocks + 2 recip-denominator broadcast blocks
                pb = pbp.tile([P, (HG * 3 + 2) * P], BF16, tag="pb", name="pb")
                den = denp.tile([P, HG], F32, tag="den", name="den")
                rec = denp.tile([P, HG], F32, tag="rec", name="rec")
                for pr in range(2):
                    s2 = s2p.tile([P, 2, 512], F32, tag="s2", name="s2")
                    s2t[(qb, pr)] = s2
                    if njb < 3:
                        # first uses of this psum bank can hold junk
                        # (incl. NaN); zero it so exp sees finite input
                        nc.vector.memset(s2, 0.0)
                    for hh in range(2):
                        p0 = hh * DK
                        nc.tensor.matmul(
                            s2[:, hh, 3 * P - jw:3 * P],
                            qt_sb[p0:p0 + DK, pr, qsl],
                            kt_sb[p0:p0 + DK, pr, j0:j0 + jw],
                            start=True, stop=True,
                        )
                    psl = slice(pr * 2 * 3 * P, (pr * 2 + 2) * 3 * P)
                    nc.scalar.activation(
                        out=pb[:, psl].rearrange("p (h n) -> p h n", h=2),
                        in_=s2[:, :, 0:3 * P],
                        func=mybir.ActivationFunctionType.Exp,
                    )
                    for hh in range(2):
                        h = pr * 2 + hh
                        hsl = slice(h * 3 * P, (h + 1) * 3 * P)
                        # pb *= bandmask01; den[h] = sum_j pb
                        nc.vector.scalar_tensor_tensor(
                            out=pb[:, hsl], in0=pb[:, hsl], scalar=1.0,
                            in1=m01_sb[:, cls, :],
                            op0=mybir.AluOpType.mult,
                            op1=mybir.AluOpType.mult,
                            accum_out=den[:, h:h + 1],
                        )
                nc.vector.reciprocal(out=rec, in_=den)
                # broadcast 1/den into two trailing blocks: after the
                # transpose they sit key-side, aligned with av's partitions
                dsl = pb[:, HG * 3 * P:].rearrange(
                    "p (z h f) -> p z h f", z=2, h=2)
                nc.gpsimd.tensor_copy(
                    dsl, rec.rearrange("p (z h) -> p z h", z=2).broadcast_to(
                        (P, 2, 2, DK)))
                pts = ptsp.tile([P, HG * 3 + 2, P], BF16, tag="pts", name="pts")
                nc.sync.dma_start_transpose(out=pts, in_=pb)
                pbt[qb] = pb
                ptst[qb] = pts
                dent[qb] = (den, rec)

            def emit_av(qb):
                """P^T @ V for query block qb into ot (feature-major)."""
                st, t = divmod(qb, 4)
                njb = min(3, qb + 1)
                pts = ptst[qb]
                avb = mmp.tile([P, 512], F32, tag="mm", name="mm")
                av = avb[:, 0:2 * P].rearrange("p (o q) -> p o q", o=2)
                avt[qb] = av
                if t == 0:
                    ott[st] = otp.tile([P, DG // P, 512], BF16, tag="ot", name="ot")
                for h in range(HG):
                    oc, hh = divmod(h, 2)
                    p0 = hh * DK
                    for i, c in enumerate(range(3 - njb, 3)):
                        jb = qb - 2 + c
                        nc.tensor.matmul(
                            av[p0:p0 + DK, oc, :],
                            v_sb[:, jb, h * DK:(h + 1) * DK],
                            pts[:, h * 3 + c, :],
                            start=(i == 0), stop=(i == njb - 1),
                            skip_group_check=True,
                        )
                nc.vector.scalar_tensor_tensor(
                    out=ott[st][:, :, t * P:(t + 1) * P],
                    in0=av, scalar=1.0, in1=pts[:, HG * 3:HG * 3 + 2, :],
                    op0=mybir.AluOpType.mult, op1=mybir.AluOpType.mult)

            def emit_d_tt(st, tt):
                """Partial out-projection for ONE token tile (spread across
                query blocks like LN)."""
                ot = ott[st]
                if True:
                    tb = st * 4 + tt
                    ob = obp.tile([P, D], BF16, tag="ob", name="ob")
                    for on in range(2):
                        ps = mmp.tile([P, 512], F32, tag="mm", name="mm")
                        for kd in range(DG // P):
                            nc.tensor.matmul(
                                ps,
                                ot[:, kd, tt * P:(tt + 1) * P],
                                wo_sb[:, kd, on * 512:(on + 1) * 512],
                                start=(kd == 0), stop=(kd == DG // P - 1),
                            )
                        if on == 0:
                            nc.scalar.copy(ob[:, 0:512], ps)
                        else:
                            nc.vector.tensor_copy(ob[:, 512:1024], ps)
                    nc.scalar.dma_start(
                        out=partial[tb * P:(tb + 1) * P, :], in_=ob)

            # ---- software-pipelined emission ----
            pending_d = []
            for qb in range(NT):
                st, t = divmod(qb, 4)
                if qb == 0:
                    emit_ln_x(0)
                    for k in range(4):
                        emit_ln_tile(0, k)
                if t == 0:
                    emit_qkv(st)
                if st < NST - 1:
                    if t == 0:
                        emit_ln_x(st + 1)
                    emit_ln_tile(st + 1, t)
                if qb >= AVLAG:
                    k = qb - AVLAG
                    emit_av(k)
                    if k % 4 == 3:
                        pending_d += [(k // 4, tt) for tt in range(4)]
                emit_qk_softmax(qb)
                if pending_d:
                    emit_d_tt(*pending_d.pop(0))
            for k in range(NT - AVLAG, NT):
                emit_av(k)
                if k % 4 == 3:
                    pending_d += [(k // 4, tt) for tt in range(4)]
                if pending_d:
                    emit_d_tt(*pending_d.pop(0))
            while pending_d:
                emit_d_tt(*pending_d.pop(0))


def build_nc():
    nc = bacc.Bacc("TRN2", target_bir_lowering=False, debug=False,
                   num_devices=8)
    with tile.TileContext(nc) as tc:
        _body(tc)
    nc.compile()
    return nc


def _prep_core_inputs(x, Wq, Wk, Wv, Wo, gamma, beta):
    """Host-side prep: per-(batch, head-group) input dicts."""
    import ml_dtypes
    BF = ml_dtypes.bfloat16
    B = x.shape[0]
    ii = np.arange(P)[:, None]
    jj = np.arange(P)[None, :]
    mup01 = (jj > ii).astype(np.float32)    # oldest block: keep strict upper
    mlo01 = (jj <= ii).astype(np.float32)   # diagonal block: keep lower+diag
    ones = np.ones((P, P), np.float32)
    zeros = np.zeros((P, P), np.float32)
    # mask class by min(qb, 2): [oldest, middle, diagonal] key blocks
    m_cls = np.stack([
        np.concatenate([zeros, zeros, mlo01], axis=1),   # qb == 0
        np.concatenate([zeros, ones, mlo01], axis=1),    # qb == 1
        np.concatenate([mup01, ones, mlo01], axis=1),    # qb >= 2
    ], axis=1)                                           # [P, 3, 384]
    m01 = np.ascontiguousarray(m_cls.reshape(P, 3 * 3 * P)).astype(BF)

    in_maps = []
    for b in range(B):
        for g in range(4):
            sl = slice(g * DG, (g + 1) * DG)
            sq = np.float32(1.0 / np.sqrt(DK))
            wq_g = (gamma[:, None] * Wq[:, sl] * sq).astype(BF)
            wk_g = (gamma[:, None] * Wk[:, sl]).astype(BF)
            wv_g = (gamma[:, None] * Wv[:, sl]).astype(BF)
            bq_g = ((beta @ Wq[:, sl]) * sq).astype(np.float32)
            bk_g = (beta @ Wk[:, sl]).astype(np.float32)
            bv_g = (beta @ Wv[:, sl]).astype(np.float32)
            in_maps.append({
                "x": np.ascontiguousarray(x[b]).astype(BF),
                "wq": wq_g, "wk": wk_g, "wv": wv_g,
                "wo": np.ascontiguousarray(Wo[sl, :]).astype(BF),
                "bq": np.ascontiguousarray(bq_g.reshape(DG // P, P).T),
                "bk": np.ascontiguousarray(bk_g.reshape(DG // P, P).T),
                "bv": np.tile(bv_g[None, :], (P, 1)),
                "m01": m01,
            })
    return in_maps


def _ntff_hook(so_path="/opt/axon/libaxon_pjrt.so"):
    import contextlib
    import ctypes

    lib = ctypes.CDLL(so_path)
    lib.axon_start_nrt_profile.argtypes = [
        ctypes.POINTER(ctypes.c_int64), ctypes.c_size_t]
    lib.axon_start_nrt_profile.restype = ctypes.c_int64
    lib.axon_stop_nrt_profile.argtypes = [ctypes.c_char_p]
    lib.axon_stop_nrt_profile.restype = ctypes.c_int64

    @contextlib.contextmanager
    def _hook(output_dir, device_ids):
        import jax
        jax.devices()
        if device_ids:
            ids = (ctypes.c_int64 * len(device_ids))(*device_ids)
            rc = lib.axon_start_nrt_profile(ids, len(device_ids))
        else:
            rc = lib.axon_start_nrt_profile(None, 0)
        if rc != 0:
            raise RuntimeError(f"axon_start_nrt_profile rc={rc}")
        try:
            yield
        finally:
            n = lib.axon_stop_nrt_profile(str(output_dir).encode())
            print(f"profile: {n} file(s) written to {output_dir}")

    return _hook


def _run_traced(nc, in_maps, trace_dir=None):
    """Execute via PJRT with NTFF capture; return BassKernelResults with
    exec_time_ns and a perfetto trace."""
    import glob
    import tempfile

    import gauge.profiler
    from concourse import bass2jax, bass_utils
    from concourse._compat import FishPath

    neff_dir = trace_dir or tempfile.mkdtemp(prefix="trn_trace_")
    hook = _ntff_hook()
    with hook(neff_dir, [0]):
        results = bass2jax.run_bass_via_pjrt(nc, in_maps, n_cores=len(in_maps))

    ntffs = glob.glob(os.path.join(neff_dir, "*_body*.ntff"))
    if not ntffs:
        print(f"no ntffs in {neff_dir}: {os.listdir(neff_dir)}")
        return bass_utils.BassKernelResults(
            results=results, instructions_and_trace=None,
            profile_json=None, exec_time_ns=None)

    profile = gauge.profiler.Profile(
        profile_path=FishPath(neff_dir),
        kernel_dev_mode=True,
        profile_on_exit=False,
        bass_kernel=nc.m,
        offline_processing=True,
        fname="*_body*",
        metadata={},
    )
    return bass_utils._process_ntff_profile(
        profile, neff_dir, nc, list(range(len(in_maps))),
        None, False, {}, trace_events=False,
    ).as_bass_kernel_results(results)


def kernel(x, Wq, Wk, Wv, Wo, bo, gamma, beta, trace=False):
    global LAST_PROFILE
    x = np.asarray(x, dtype=np.float32)
    Wq, Wk, Wv, Wo = (np.asarray(a, dtype=np.float32) for a in (Wq, Wk, Wv, Wo))
    bo = np.asarray(bo, dtype=np.float32)
    gamma = np.asarray(gamma, dtype=np.float32)
    beta = np.asarray(beta, dtype=np.float32)

    nc = build_nc()
    in_maps = _prep_core_inputs(x, Wq, Wk, Wv, Wo, gamma, beta)
    if trace:
        res = _run_traced(nc, in_maps)
    else:
        res = run_bass_kernel_spmd(nc, in_maps, core_ids=list(range(8)))
    LAST_PROFILE = {"exec_time_ns": res.exec_time_ns}

    B = x.shape[0]
    out = np.empty_like(x)
    for b in range(B):
        acc = x[b] + bo[None, :]
        for g in range(4):
            acc = acc + res.results[b * 4 + g]["partial"].astype(np.float32)
        out[b] = acc
    return out


# revision 20
# speedup vs baseline: 1.0087x; 1.0087x over previous
"""Local causal (sliding-window) attention block on 8 TRN2 NeuronCores.

Reference computation (per batch b):
    h = LayerNorm(x) * gamma + beta
    Q = h@Wq, K = h@Wk, V = h@Wv          (heads: 16 x 64)
    S = QK^T/sqrt(dk) masked to causal band of width 256
    out = x + softmax(S)@V @ Wo + bo

Sharding: 8 cores = 2 batches x 4 head-groups (4 heads each).
Each core computes LN(x_b), its head-group's Q/K/V, banded attention,
and a partial out-projection  attn_g @ Wo[g]  (token-major, [T, D]).
Host reduces: out[b] = x[b] + sum_g partial[b,g] + bo.

Pipelined single-pass design:
  - all matmul operands bf16 (1 cyc/row on PE, FWL weight loads)
  - h^T and P^T via XBAR DMA-transpose (no PE transposes, no PSUM
    evacuation copies for transposes)
  - band mask applied multiplicatively after exp on GpSimd via
    scalar_tensor_tensor with fused denominator accumulation
  - per-supertile (512 tokens) software pipeline: LN -> QKV -> banded
    attention (lag-3 between softmax and AV) -> out-projection, so the
    PE never idles long enough to lose the HAM clock-gate warmup.
"""

import os

import numpy as np

import concourse.bass as bass
import concourse.tile as tile
from concourse import bacc, mybir
from concourse.bass_utils import run_bass_kernel_spmd

F32 = mybir.dt.float32
BF16 = mybir.dt.bfloat16

T = 2048          # tokens per batch
D = 1024          # model dim
HG = 4            # heads per core
DK = 64           # head dim
DG = HG * DK      # head-group feature width (256)
WIN = 256         # attention window
P = 128           # partitions
NT = T // P       # 16 token tiles
NST = 4           # supertiles (512 tokens each)
KC = D // P       # 8 feature chunks
LN_EPS = 1e-5
AVLAG = 6         # softmax->AV pipeline lag in query blocks

# filled by test.py via run(trace=True)
LAST_PROFILE = {}


def _body(tc):
    nc = tc.nc

    x = nc.dram_tensor("x", [T, D], BF16, kind="ExternalInput").ap()
    wq = nc.dram_tensor("wq", [D, DG], BF16, kind="ExternalInput").ap()
    wk = nc.dram_tensor("wk", [D, DG], BF16, kind="ExternalInput").ap()
    wv = nc.dram_tensor("wv", [D, DG], BF16, kind="ExternalInput").ap()
    wo = nc.dram_tensor("wo", [DG, D], BF16, kind="ExternalInput").ap()
    bq = nc.dram_tensor("bq", [P, DG // P], F32, kind="ExternalInput").ap()
    bk = nc.dram_tensor("bk", [P, DG // P], F32, kind="ExternalInput").ap()
    bv = nc.dram_tensor("bv", [P, DG], F32, kind="ExternalInput").ap()
    m01 = nc.dram_tensor("m01", [P, 3 * 3 * P], BF16, kind="ExternalInput").ap()
    partial = nc.dram_tensor("partial", [T, D], BF16, kind="ExternalOutput").ap()

    with (
        tc.tile_pool(name="consts", bufs=1) as consts,
        tc.tile_pool(name="big", bufs=1) as big,
    ):
        # ---- resident SBUF tensors ----
        wq_sb = consts.tile([P, KC, DG], BF16, tag="wq")
        wk_sb = consts.tile([P, KC, DG], BF16, tag="wk")
        wv_sb = consts.tile([P, KC, DG], BF16, tag="wv")
        wo_sb = consts.tile([P, DG // P, D], BF16, tag="wo")
        bq_sb = consts.tile([P, DG // P], F32, tag="bq")
        bk_sb = consts.tile([P, DG // P], F32, tag="bk")
        bv_sb = consts.tile([P, DG], F32, tag="bv")
        ones1_sb = consts.tile([1, P], BF16, tag="ones1")
        bv1_sb = consts.tile([1, DG], BF16, tag="bv1")
        m01_sb = consts.tile([P, 3, 3 * P], BF16, tag="m01")
        eps_sb = consts.tile([P, 1], F32, tag="eps")

        nc.sync.dma_start(out=wv_sb, in_=wv.rearrange("(c p) n -> p c n", p=P))
        nc.sync.dma_start(out=wq_sb, in_=wq.rearrange("(c p) n -> p c n", p=P))
        nc.sync.dma_start(out=wk_sb, in_=wk.rearrange("(c p) n -> p c n", p=P))
        nc.scalar.dma_start(out=wo_sb, in_=wo.rearrange("(c p) n -> p c n", p=P))
        nc.sync.dma_start(out=bq_sb, in_=bq)
        nc.sync.dma_start(out=bk_sb, in_=bk)
        nc.sync.dma_start(out=bv_sb, in_=bv)
        nc.sync.dma_start(out=m01_sb, in_=m01.rearrange("p (c n) -> p c n", c=3))
        nc.vector.memset(eps_sb, LN_EPS)
        nc.vector.memset(ones1_sb, 1.0)
        nc.vector.tensor_copy(bv1_sb, bv_sb[0:1, :])

        # feature-major Q^T/K^T (scaled; bias folded), token-major V,
        # all bf16, resident for the whole batch
        qt_sb = big.tile([P, DG // P, T], BF16, tag="qt")
        kt_sb = big.tile([P, DG // P, T], BF16, tag="kt")
        v_sb = big.tile([P, NT, DG], BF16, tag="v")

        with (
            tc.tile_pool(name="xp", bufs=2) as xp,
            tc.tile_pool(name="hnp", bufs=4) as hnp,
            tc.tile_pool(name="lnst", bufs=4) as lnst,
            tc.tile_pool(name="rsp", bufs=8) as rsp,
            tc.tile_pool(name="htp", bufs=2) as htp,
            tc.tile_pool(name="otp", bufs=2) as otp,
            tc.tile_pool(name="obp", bufs=2) as obp,
            tc.tile_pool(name="pbp", bufs=5) as pbp,
            tc.tile_pool(name="ptsp", bufs=8) as ptsp,
            tc.tile_pool(name="denp", bufs=6) as denp,
            tc.tile_pool(name="s2p", bufs=3, space="PSUM") as s2p,
            tc.tile_pool(name="mmp", bufs=2, space="PSUM") as mmp,
        ):
            ht = {}      # st -> ht tile [P, 4, KC, P]
            s2t = {}     # (qb, pair) -> psum tile
            pbt = {}     # qb -> [P, 4*3*P] bf16
            ptst = {}    # qb -> [P, 12, P] bf16
            dent = {}    # qb -> ([P,4] den, [P,4] recip)
            avt = {}     # qb -> [P, 2, P] psum
            ott = {}     # st -> [P, 2, 512] bf16

            x4t = {}

            def emit_ln_x(st):
                """Allocate ht + load the supertile's x in one DMA."""
                ht[st] = htp.tile([P, 4, KC, P], BF16, tag="ht", name="ht")
                x4 = xp.tile([P, 4, D], BF16, tag="x4", name="x4")
                x4t[st] = x4
                nc.scalar.dma_start(
                    out=x4, in_=x.rearrange("(s t p) d -> p (s t) d", p=P, t=4)[
                        :, st * 4:(st + 1) * 4, :])

            def emit_ln_tile(st, t):
                """LN stats + apply + h^T transpose for ONE token tile, so
                LN work spreads across query blocks and never bunches up
                ahead of the softmax exps in the ACT queue."""
                if True:
                    tb = st * 4 + t
                    xt = x4t[st][:, t, :]

                    stats = lnst.tile([P, 2, 6], F32, tag="stats")
                    xg = xt.rearrange("p (g d) -> p g d", g=2)
                    nc.vector.bn_stats(out=stats[:, 0, :], in_=xg[:, 0, :])
                    nc.vector.bn_stats(out=stats[:, 1, :], in_=xg[:, 1, :])
                    mv = lnst.tile([P, 2], F32, tag="mv")
                    nc.vector.bn_aggr(out=mv, in_=stats)

                    rstd = rsp.tile([P, 1], F32, tag="rstd", name="rstd")
                    nc.scalar.activation(
                        out=rstd, in_=mv[:, 1:2],
                        func=mybir.ActivationFunctionType.Sqrt,
                        bias=eps_sb, scale=1.0,
                    )
                    nc.vector.reciprocal(out=rstd, in_=rstd)
                    nmr = rsp.tile([P, 1], F32, tag="nmr", name="nmr")
                    nc.vector.scalar_tensor_tensor(
                        out=nmr, in0=mv[:, 0:1], scalar=-1.0, in1=rstd,
                        op0=mybir.AluOpType.mult, op1=mybir.AluOpType.mult)

                    hn = hnp.tile([P, D], BF16, tag="hn")
                    nc.scalar.activation(
                        out=hn, in_=xt,
                        func=mybir.ActivationFunctionType.Identity,
                        bias=nmr, scale=rstd,
                    )
                    nc.sync.dma_start_transpose(out=ht[st][:, t, :, :], in_=hn)

            def emit_qkv(st):
                """Q/K (feature-major) and V (token-major) projections for
                supertile st from ht[st]."""
                h = ht[st]
                tsl = slice(st * 512, (st + 1) * 512)
                for t in range(4):
                    tb = st * 4 + t
                    ps = mmp.tile([P, 512], F32, tag="mm", name="mm")
                    for kc in range(KC):
                        nc.tensor.matmul(
                            ps[:, 0:DG],
                            h[:, t, kc, :],
                            wv_sb[:, kc, :],
                            start=(kc == 0), stop=False,
                        )
                    # rank-1 bias: ps += ones_col.T @ bv_row
                    nc.tensor.matmul(
                        ps[:, 0:DG], ones1_sb, bv1_sb,
                        start=False, stop=True,
                    )
                    nc.vector.tensor_copy(v_sb[:, tb, :], ps[:, 0:DG])
                for w_sb, dst, b_sb in ((wq_sb, qt_sb, bq_sb),
                                        (wk_sb, kt_sb, bk_sb)):
                    for oc in range(DG // P):
                        ps = mmp.tile([P, 512], F32, tag="mm", name="mm")
                        for kc in range(KC):
                            nc.tensor.matmul(
                                ps,
                                w_sb[:, kc, oc * P:(oc + 1) * P],
                                h[:, :, kc, :],
                                start=(kc == 0), stop=(kc == KC - 1),
                            )
                        nc.vector.tensor_scalar_add(
                            dst[:, oc, tsl], ps, b_sb[:, oc:oc + 1])

            def emit_qk_softmax(qb):
                """Banded QK^T, exp, multiplicative band mask + denominator
                (GpSimd), normalization, and DMA-transpose of P."""
                njb = min(3, qb + 1)
                jw = njb * P
                j0 = (qb - njb + 1) * P
                cls = min(qb, 2)
                qsl = slice(qb * P, (qb + 1) * P)
                # 12 P blocks + 2 recip-denominator broadcast blocks
                pb = pbp.tile([P, (HG * 3 + 2) * P], BF16, tag="pb", name="pb")
                den = denp.tile([P, HG], F32, tag="den", name="den")
                rec = denp.tile([P, HG], F32, tag="rec", name="rec")
                for pr in range(2):
                    s2 = s2p.tile([P, 2, 512], F32, tag="s2", name="s2")
                    s2t[(qb, pr)] = s2
                    if njb < 3:
                        # first uses of this psum bank can hold junk
                        # (incl. NaN); zero it so exp sees finite input
                        nc.vector.memset(s2, 0.0)
                    for hh in range(2):
                        p0 = hh * DK
                        nc.tensor.matmul(
                            s2[:, hh, 3 * P - jw:3 * P],
                            qt_sb[p0:p0 + DK, pr, qsl],
                            kt_sb[p0:p0 + DK, pr, j0:j0 + jw],
                            start=True, stop=True,
                        )
                    psl = slice(pr * 2 * 3 * P, (pr * 2 + 2) * 3 * P)
                    nc.scalar.activation(
                        out=pb[:, psl].rearrange("p (h n) -> p h n", h=2),
                        in_=s2[:, :, 0:3 * P],
                        func=mybir.ActivationFunctionType.Exp,
                    )
                    for hh in range(2):
                        h = pr * 2 + hh
                        hsl = slice(h * 3 * P, (h + 1) * 3 * P)
                        # pb *= bandmask01; den[h] = sum_j pb
                        nc.vector.scalar_tensor_tensor(
                            out=pb[:, hsl], in0=pb[:, hsl], scalar=1.0,
                            in1=m01_sb[:, cls, :],
                            op0=mybir.AluOpType.mult,
                            op1=mybir.AluOpType.mult,
                            accum_out=den[:, h:h + 1],
                        )
                nc.vector.reciprocal(out=rec, in_=den)
                # broadcast 1/den into two trailing blocks: after the
                # transpose they sit key-side, aligned with av's partitions
                dsl = pb[:, HG * 3 * P:].rearrange(
                    "p (z h f) -> p z h f", z=2, h=2)
                nc.gpsimd.tensor_copy(
                    dsl, rec.rearrange("p (z h) -> p z h", z=2).broadcast_to(
                        (P, 2, 2, DK)))
                pts = ptsp.tile([P, HG * 3 + 2, P], BF16, tag="pts", name="pts")
                nc.sync.dma_start_transpose(out=pts, in_=pb)
                pbt[qb] = pb
                ptst[qb] = pts
                dent[qb] = (den, rec)

            def emit_av(qb):
                """P^T @ V for query block qb into ot (feature-major)."""
                st, t = divmod(qb, 4)
                njb = min(3, qb + 1)
                pts = ptst[qb]
                avb = mmp.tile([P, 512], F32, tag="mm", name="mm")
                av = avb[:, 0:2 * P].rearrange("p (o q) -> p o q", o=2)
                avt[qb] = av
                if t == 0:
                    ott[st] = otp.tile([P, DG // P, 512], BF16, tag="ot", name="ot")
                for h in range(HG):
                    oc, hh = divmod(h, 2)
                    p0 = hh * DK
                    for i, c in enumerate(range(3 - njb, 3)):
                        jb = qb - 2 + c
                        nc.tensor.matmul(
                            av[p0:p0 + DK, oc, :],
                            v_sb[:, jb, h * DK:(h + 1) * DK],
                            pts[:, h * 3 + c, :],
                            start=(i == 0), stop=(i == njb - 1),
                            skip_group_check=True,
                        )
                nc.vector.scalar_tensor_tensor(
                    out=ott[st][:, :, t * P:(t + 1) * P],
                    in0=av, scalar=1.0, in1=pts[:, HG * 3:HG * 3 + 2, :],
                    op0=mybir.AluOpType.mult, op1=mybir.AluOpType.mult)

            def emit_d_tt(st, tt):
                """Partial out-projection for ONE token tile (spread across
                query blocks like LN)."""
                ot = ott[st]
                if True:
                    tb = st * 4 + tt
                    ob = obp.tile([P, D], BF16, tag="ob", name="ob")
                    for on in range(2):
                        ps = mmp.tile([P, 512], F32, tag="mm", name="mm")
                        for kd in range(DG // P):
                            nc.tensor.matmul(
                                ps,
                                ot[:, kd, tt * P:(tt + 1) * P],
                                wo_sb[:, kd, on * 512:(on + 1) * 512],
                                start=(kd == 0), stop=(kd == DG // P - 1),
                            )
                        if on == 0:
                            nc.scalar.copy(ob[:, 0:512], ps)
                        else:
                            nc.vector.tensor_copy(ob[:, 512:1024], ps)
                    nc.sync.dma_start(
                        out=partial[tb * P:(tb + 1) * P, :], in_=ob)

            # ---- software-pipelined emission ----
            pending_d = []
            for qb in range(NT):
                st, t = divmod(qb, 4)
                if qb == 0:
                    emit_ln_x(0)
                    for k in range(4):
                        emit_ln_tile(0, k)
                if t == 0:
                    emit_qkv(st)
                if st < NST - 1:
                    if t == 0:
                        emit_ln_x(st + 1)
                    emit_ln_tile(st + 1, t)
                if qb >= AVLAG:
                    k = qb - AVLAG
                    emit_av(k)
                    if k % 4 == 3:
                        pending_d += [(k // 4, tt) for tt in range(4)]
                emit_qk_softmax(qb)
                if pending_d:
                    emit_d_tt(*pending_d.pop(0))
            for k in range(NT - AVLAG, NT):
                emit_av(k)
                if k % 4 == 3:
                    pending_d += [(k // 4, tt) for tt in range(4)]
                if pending_d:
                    emit_d_tt(*pending_d.pop(0))
            while pending_d:
                emit_d_tt(*pending_d.pop(0))


def build_nc():
    nc = bacc.Bacc("TRN2", target_bir_lowering=False, debug=False,
                   num_devices=8)
    with tile.TileContext(nc) as tc:
        _body(tc)
    nc.compile()
    return nc


def _prep_core_inputs(x, Wq, Wk, Wv, Wo, gamma, beta):
    """Host-side prep: per-(batch, head-group) input dicts."""
    import ml_dtypes
    BF = ml_dtypes.bfloat16
    B = x.shape[0]
    ii = np.arange(P)[:, None]
    jj = np.arange(P)[None, :]
    mup01 = (jj > ii).astype(np.float32)    # oldest block: keep strict upper
    mlo01 = (jj <= ii).astype(np.float32)   # diagonal block: keep lower+diag
    ones = np.ones((P, P), np.float32)
    zeros = np.zeros((P, P), np.float32)
    # mask class by min(qb, 2): [oldest, middle, diagonal] key blocks
    m_cls = np.stack([
        np.concatenate([zeros, zeros, mlo01], axis=1),   # qb == 0
        np.concatenate([zeros, ones, mlo01], axis=1),    # qb == 1
        np.concatenate([mup01, ones, mlo01], axis=1),    # qb >= 2
    ], axis=1)                                           # [P, 3, 384]
    m01 = np.ascontiguousarray(m_cls.reshape(P, 3 * 3 * P)).astype(BF)

    in_maps = []
    for b in range(B):
        for g in range(4):
            sl = slice(g * DG, (g + 1) * DG)
            sq = np.float32(1.0 / np.sqrt(DK))
            wq_g = (gamma[:, None] * Wq[:, sl] * sq).astype(BF)
            wk_g = (gamma[:, None] * Wk[:, sl]).astype(BF)
            wv_g = (gamma[:, None] * Wv[:, sl]).astype(BF)
            bq_g = ((beta @ Wq[:, sl]) * sq).astype(np.float32)
            bk_g = (beta @ Wk[:, sl]).astype(np.float32)
            bv_g = (beta @ Wv[:, sl]).astype(np.float32)
            in_maps.append({
                "x": np.ascontiguousarray(x[b]).astype(BF),
                "wq": wq_g, "wk": wk_g, "wv": wv_g,
                "wo": np.ascontiguousarray(Wo[sl, :]).astype(BF),
                "bq": np.ascontiguousarray(bq_g.reshape(DG // P, P).T),
                "bk": np.ascontiguousarray(bk_g.reshape(DG // P, P).T),
                "bv": np.tile(bv_g[None, :], (P, 1)),
                "m01": m01,
            })
    return in_maps


def _ntff_hook(so_path="/opt/axon/libaxon_pjrt.so"):
    import contextlib
    import ctypes

    lib = ctypes.CDLL(so_path)
    lib.axon_start_nrt_profile.argtypes = [
        ctypes.POINTER(ctypes.c_int64), ctypes.c_size_t]
    lib.axon_start_nrt_profile.restype = ctypes.c_int64
    lib.axon_stop_nrt_profile.argtypes = [ctypes.c_char_p]
    lib.axon_stop_nrt_profile.restype = ctypes.c_int64

    @contextlib.contextmanager
    def _hook(output_dir, device_ids):
        import jax
        jax.devices()
        if device_ids:
            ids = (ctypes.c_int64 * len(device_ids))(*device_ids)
            rc = lib.axon_start_nrt_profile(ids, len(device_ids))
        else:
            rc = lib.axon_start_nrt_profile(None, 0)
        if rc != 0:
            raise RuntimeError(f"axon_start_nrt_profile rc={rc}")
        try:
            yield
        finally:
            n = lib.axon_stop_nrt_profile(str(output_dir).encode())
            print(f"profile: {n} file(s) written to {output_dir}")

    return _hook


def _run_traced(nc, in_maps, trace_dir=None):
    """Execute via PJRT with NTFF capture; return BassKernelResults with
    exec_time_ns and a perfetto trace."""
    import glob
    import tempfile

    import gauge.profiler
    from concourse import bass2jax, bass_utils
    from concourse._compat import FishPath

    neff_dir = trace_dir or tempfile.mkdtemp(prefix="trn_trace_")
    hook = _ntff_hook()
    with hook(neff_dir, [0]):
        results = bass2jax.run_bass_via_pjrt(nc, in_maps, n_cores=len(in_maps))

    ntffs = glob.glob(os.path.join(neff_dir, "*_body*.ntff"))
    if not ntffs:
        print(f"no ntffs in {neff_dir}: {os.listdir(neff_dir)}")
        return bass_utils.BassKernelResults(
            results=results, instructions_and_trace=None,
            profile_json=None, exec_time_ns=None)

    profile = gauge.profiler.Profile(
        profile_path=FishPath(neff_dir),
        kernel_dev_mode=True,
        profile_on_exit=False,
        bass_kernel=nc.m,
        offline_processing=True,
        fname="*_body*",
        metadata={},
    )
    return bass_utils._process_ntff_profile(
        profile, neff_dir, nc, list(range(len(in_maps))),
        None, False, {}, trace_events=False,
    ).as_bass_kernel_results(results)


def kernel(x, Wq, Wk, Wv, Wo, bo, gamma, beta, trace=False):
    global LAST_PROFILE
    x = np.asarray(x, dtype=np.float32)
    Wq, Wk, Wv, Wo = (np.asarray(a, dtype=np.float32) for a in (Wq, Wk, Wv, Wo))
    bo = np.asarray(bo, dtype=np.float32)
    gamma = np.asarray(gamma, dtype=np.float32)
    beta = np.asarray(beta, dtype=np.float32)

    nc = build_nc()
    in_maps = _prep_core_inputs(x, Wq, Wk, Wv, Wo, gamma, beta)
    if trace:
        res = _run_traced(nc, in_maps)
    else:
        res = run_bass_kernel_spmd(nc, in_maps, core_ids=list(range(8)))
    LAST_PROFILE = {"exec_time_ns": res.exec_time_ns}

    B = x.shape[0]
    out = np.empty_like(x)
    for b in range(B):
        acc = x[b] + bo[None, :]
        for g in range(4):
            acc = acc + res.results[b * 4 + g]["partial"].astype(np.float32)
        out[b] = acc
    return out
